# revision 1
# baseline (speedup 1.0000x reference)
"""CopyGenerator kernel for 8 trn2 NeuronCores.

Strategy (vocab tensor-parallel):
  - W's vocab dim (50000) is sharded 6250 cols/core, padded to 6656 = 13*512.
  - Per core: logits = hidden @ W_shard via PE (fp16 operands, fp32 PSUM
    accumulate), exp via ACT with fused row-sum accumulation (fp16 exp kept
    in SBUF), softmax denominator completed with one tiny AllReduce per
    row-chunk, then one scale pass applies (1-copy)/Z and streams out.
  - Rows processed in 2 uneven chunks (1280 + 768) so the fp16 exp buffer
    fits SBUF and the non-overlappable final scale pass is short.
  - DMA issue is spread across engine queues: W loads on sync, hT/small
    loads on vector, stores on gpsimd.
  - copy gate sigmoid(hidden@w_copy) and the attn x src_map einsum run on
    the same cores (tiny); host takes core 0's copy region.
PAD col handling: host zeroes W[:,1] on core 0, kernel masks the exp column
and subtracts the constant exp(0)=1 from that core's row sums.
"""

import numpy as np

N, D, V = 2048, 1024, 50000
S, B, CV = 100, 32, 120
NCORES = 8
VREAL = 6250          # real vocab cols per core
VSH = 6656            # padded (13 * 512)
VT = 13               # v-tiles of 512
VTAIL = VREAL - 12 * 512   # 106 real cols in last v-tile
KT = 8                # k-tiles of 128 over D
CHUNKS = [(0, 10), (10, 6)]   # (first n-tile, n-tile count); 16 x 128 rows
PAD_IDX = 1

_CACHE = {}
TRACE = False


def _install_walrus_compat():
    """This container's walrus build rejects >1 sync-wait per instruction.
    Patch the Tile drain to chain single-wait drains, and provide a module
    post-pass hoisting extra waits onto same-engine NoOps."""
    import concourse.tile as tile_mod
    import concourse.mybir as mybir
    from concourse.vector_clock import ScopedClock

    if getattr(tile_mod.TileContext._drain_and_barrier, "_waitsplit", False):
        return

    def _patched_drain_and_barrier(self, tick_clock, wait_clock):
        nc = self.nc
        drain_inst = nc.sync.drain()
        wait_clock.add_sem_waits(
            drain_inst.ins, ScopedClock({None: tick_clock.global_clock})
        )
        si = drain_inst.ins.sync_info
        waits = list(si.on_wait) if si and si.on_wait else []
        if len(waits) > 1:
            si.on_wait = waits[:1]
            rest = waits[1:]
            while rest:
                chunk, rest = rest[:1], rest[1:]
                d2 = nc.sync.drain()
                if d2.ins.sync_info is None:
                    d2.ins.sync_info = mybir.SyncInfo(on_wait=chunk, on_update=[])
                else:
                    d2.ins.sync_info.on_wait = chunk
        nc.all_engine_barrier()
        assert self.sems is not None
        popped = nc._tile_sem_poison_stack.pop()
        assert popped is self._sem_poison
        nc.clear_and_free_semaphores(list(self.sems.allocated().values()))
        nc.all_engine_barrier()

    _patched_drain_and_barrier._waitsplit = True
    tile_mod.TileContext._drain_and_barrier = _patched_drain_and_barrier


def _split_multi_waits(nc):
    import concourse.mybir as mybir

    uid = 0
    n_split = 0
    for fn in nc.m.functions:
        for bb in fn.blocks:
            old = list(bb.instructions)
            new = []
            changed = False
            for ins in old:
                si = ins.sync_info
                waits = list(si.on_wait) if si and si.on_wait else []
                if len(waits) > 1:
                    changed = True
                    n_split += 1
                    for w in waits[:-1]:
                        uid += 1
                        new.append(
                            mybir.InstNoOp(
                                name=f"I-waitsplit-{uid}-{ins.name}",
                                sync_info=mybir.SyncInfo(on_wait=[w], on_update=[]),
                                bass_nofuse=True,
                                engine=ins.engine,
                            )
                        )
                    si.on_wait = [waits[-1]]
                new.append(ins)
            if changed:
                bb.instructions[:] = new
    return n_split


def _spans(total, step=512):
    o = 0
    while o < total:
        w = min(step, total - o)
        yield o, w
        o += w


def _build_nc():
    import concourse.bass as bass
    import concourse.mybir as mybir
    import concourse.tile as tile

    _install_walrus_compat()

    f32 = mybir.dt.float32
    f16 = mybir.dt.float16
    AF = mybir.ActivationFunctionType
    OP = mybir.AluOpType
    AX = mybir.AxisListType

    nc = bass.Bass()
    hT = nc.dram_tensor("hT", [D, N], f16, kind="ExternalInput")
    Wsh = nc.dram_tensor("Wsh", [D, VSH], f16, kind="ExternalInput")
    attnT = nc.dram_tensor("attnT", [S, N], f16, kind="ExternalInput")
    smap = nc.dram_tensor("smap", [S, B * CV], f16, kind="ExternalInput")
    wcp = nc.dram_tensor("wcp", [128, KT], f16, kind="ExternalInput")
    bcp = nc.dram_tensor("bcp", [1, 1], f32, kind="ExternalInput")
    cmask = nc.dram_tensor("cmask", [128, 512], f16, kind="ExternalInput")
    zcorr = nc.dram_tensor("zcorr", [128, 1], f32, kind="ExternalInput")
    out = nc.dram_tensor("out", [N, VREAL + CV], f32, kind="ExternalOutput")

    MAXNT = max(cnt for _, cnt in CHUNKS)

    with tile.TileContext(nc) as tc:
        with (
            tc.tile_pool(name="htp", bufs=8) as htp,
            tc.tile_pool(name="wp", bufs=16) as wp,
            tc.tile_pool(name="expp", bufs=MAXNT) as expp,
            tc.tile_pool(name="stgp", bufs=6) as stgp,
            tc.tile_pool(name="zpp", bufs=2 * MAXNT) as zpp,
            tc.tile_pool(name="smallp", bufs=1) as smallp,
            tc.tile_pool(name="psmain", bufs=6, space="PSUM") as psmain,
            tc.tile_pool(name="psaux", bufs=2, space="PSUM") as psaux,
            tc.tile_pool(name="dramp", bufs=1, space="DRAM") as dramp,
        ):
            # ---- persistent small tiles (vector-queue loads) ----
            wcp_sb = smallp.tile([128, KT], f16)
            nc.scalar.dma_start(wcp_sb[:], wcp[:])
            bcp_sb = smallp.tile([1, 1], f32)
            nc.scalar.dma_start(bcp_sb[:], bcp[:])
            cmask_sb = smallp.tile([128, 512], f16)
            nc.scalar.dma_start(cmask_sb[:], cmask[:])
            zcorr_sb = smallp.tile([128, 1], f32)
            nc.scalar.dma_start(zcorr_sb[:], zcorr[:])
            ones1 = smallp.tile([1, 1], f32)
            nc.vector.memset(ones1[:], 1.0)
            ones128 = smallp.tile([1, 128], f32)
            nc.vector.memset(ones128[:], 1.0)
            cg_sb = smallp.tile([1, N], f32)
            cgT = smallp.tile([128, 16], f32)

            zin = [
                dramp.tile([128, cnt], f32, name=f"zin{ci}")
                for ci, (_, cnt) in enumerate(CHUNKS)
            ]
            zout = [
                dramp.tile([128, cnt], f32, addr_space="Shared", name=f"zout{ci}")
                for ci, (_, cnt) in enumerate(CHUNKS)
            ]

            for ci, (t0, NTC) in enumerate(CHUNKS):
                ncols = NTC * 128          # rows of this chunk
                c0 = t0 * 128              # first row
                # ---- hidden^T chunk (d-major k-tiles) ----
                ht = []
                for k in range(KT):
                    t_ = htp.tile(
                        [128, MAXNT * 128], f16, tag="ht", name=f"ht{ci}_{k}"
                    )
                    nc.scalar.dma_start(
                        t_[:, 0:ncols],
                        hT[k * 128 : (k + 1) * 128, c0 : c0 + ncols],
                    )
                    ht.append(t_)

                # ---- copy gate for this chunk's rows ----
                for lo, w in _spans(ncols):
                    pg = psaux.tile([1, 512], f32, tag="psaux", name=f"pg{ci}_{lo}")
                    for k in range(KT):
                        nc.tensor.matmul(
                            pg[:, 0:w],
                            wcp_sb[:, k : k + 1],
                            ht[k][:, lo : lo + w],
                            start=(k == 0),
                            stop=(k == KT - 1),
                        )
                    nc.scalar.activation(
                        cg_sb[0:1, c0 + lo : c0 + lo + w], pg[:, 0:w], AF.Sigmoid,
                        bias=bcp_sb[0:1, 0:1],
                    )
                # transpose gate to per-partition layout [128, n-tile]
                for t in range(NTC):
                    pt = psaux.tile([128, 1], f32, tag="psaux", name=f"pt{ci}_{t}")
                    i0 = (t0 + t) * 128
                    nc.tensor.matmul(
                        pt[:], cg_sb[0:1, i0 : i0 + 128], ones1[0:1, 0:1],
                        start=True, stop=True,
                    )
                    nc.vector.tensor_copy(cgT[:, t0 + t : t0 + t + 1], pt[:])

                # ---- main matmul + exp + rowsum ----
                exps = [
                    expp.tile([128, VREAL], f16, tag="exp", name=f"exp{ci}_{t}")
                    for t in range(NTC)
                ]
                zparts = [
                    zpp.tile([128, VT], f32, tag="zpart", name=f"zp{ci}_{t}")
                    for t in range(NTC)
                ]
                for g in range(VT):
                    wt = []
                    for k in range(KT):
                        w_ = wp.tile(
                            [128, 512], f16, tag="wt", name=f"wt{ci}_{g}_{k}"
                        )
                        nc.sync.dma_start(
                            w_[:],
                            Wsh[k * 128 : (k + 1) * 128, g * 512 : (g + 1) * 512],
                        )
                        wt.append(w_)
                    for t in range(NTC):
                        pm = psmain.tile(
                            [128, 512], f32, tag="psmain", name=f"pm{ci}_{g}_{t}"
                        )
                        for k in range(KT):
                            nc.tensor.matmul(
                                pm[:],
                                ht[k][:, t * 128 : (t + 1) * 128],
                                wt[k][:],
                                start=(k == 0),
                                stop=(k == KT - 1),
                            )
                        if g < VT - 1:
                            nc.scalar.activation(
                                exps[t][:, g * 512 : (g + 1) * 512], pm[:], AF.Exp,
                                accum_out=zparts[t][:, g : g + 1],
                            )
                        else:
                            nc.scalar.activation(
                                exps[t][:, 6144:VREAL], pm[:, 0:VTAIL], AF.Exp,
                                accum_out=zparts[t][:, g : g + 1],
                            )
                        if g == 0:
                            # zero masked cols (PAD on core 0; all-ones elsewhere)
                            nc.vector.tensor_tensor(
                                exps[t][:, 0:512], exps[t][:, 0:512], cmask_sb[:],
                                OP.mult,
                            )

                # ---- denominator: reduce partials, AllReduce across cores ----
                zsum = smallp.tile([128, NTC], f32, name=f"zsum{ci}")
                for t in range(NTC):
                    nc.vector.tensor_reduce(
                        zsum[:, t : t + 1], zparts[t][:, 0:VT], axis=AX.X, op=OP.add
                    )
                nc.vector.tensor_scalar(
                    zsum[:], zsum[:], zcorr_sb[:], None, OP.subtract
                )
                nc.gpsimd.dma_start(zin[ci][:], zsum[:])
                nc.gpsimd.collective_compute(
                    "AllReduce",
                    OP.add,
                    ins=[zin[ci].opt()],
                    outs=[zout[ci].opt()],
                    replica_groups=[list(range(NCORES))],
                )
                zr = smallp.tile([128, NTC], f32, name=f"zr{ci}")
                nc.scalar.dma_start(zr[:], zout[ci][:])
                rz = smallp.tile([128, NTC], f32, name=f"rz{ci}")
                nc.vector.reciprocal(rz[:], zr[:])
                om = smallp.tile([128, NTC], f32, name=f"om{ci}")
                nc.vector.tensor_scalar(
                    om[:], cgT[:, t0 : t0 + NTC], -1.0, 1.0, OP.mult, OP.add
                )
                sc = smallp.tile([128, NTC], f32, name=f"sc{ci}")
                nc.vector.tensor_tensor(sc[:], om[:], rz[:], OP.mult)

                # ---- pass 2: scale + store (split across DVE and ACT) ----
                for t in range(NTC):
                    r0 = (t0 + t) * 128
                    for j in range(VT):
                        wdt = 512 if j < VT - 1 else VTAIL
                        col0 = j * 512 if j < VT - 1 else 6144
                        stg = stgp.tile(
                            [128, 512], f32, tag="stg", name=f"stg{ci}_{t}_{j}"
                        )
                        if j % 2 == 0:
                            nc.vector.tensor_scalar(
                                stg[:, 0:wdt],
                                exps[t][:, col0 : col0 + wdt],
                                sc[:, t : t + 1],
                                None,
                                OP.mult,
                            )
                        else:
                            nc.scalar.activation(
                                stg[:, 0:wdt],
                                exps[t][:, col0 : col0 + wdt],
                                AF.Copy,
                                scale=sc[:, t : t + 1],
                            )
                        nc.gpsimd.dma_start(
                            out[r0 : r0 + 128, col0 : col0 + wdt], stg[:, 0:wdt]
                        )

            # ---- copy path: copy_prob = einsum(attn*copy, src_map) ----
            attnT_sb = smallp.tile([128, N], f16)
            nc.scalar.dma_start(attnT_sb[0:S, :], attnT[:, :])
            smap_sb = smallp.tile([128, B * CV], f16)
            nc.scalar.dma_start(smap_sb[0:S, :], smap[:, :])
            bc_sb = smallp.tile([128, N], f16)
            for q in range(4):
                pb = psaux.tile([128, 512], f32, tag="psaux", name=f"pb{q}")
                nc.tensor.matmul(
                    pb[:], ones128[0:1, :], cg_sb[0:1, q * 512 : (q + 1) * 512],
                    start=True, stop=True,
                )
                nc.vector.tensor_copy(bc_sb[:, q * 512 : (q + 1) * 512], pb[:])
            mulT = smallp.tile([128, N], f16)
            nc.vector.tensor_tensor(
                mulT[0:S, :], attnT_sb[0:S, :], bc_sb[0:S, :], OP.mult
            )
            mulT_r = mulT.rearrange("p (t b) -> p b t", b=B)
            out_r = out[:, :].rearrange("(t b) v -> b t v", b=B)
            for bb_ in range(B):
                pc = psaux.tile([64, CV], f32, tag="psaux", name=f"pc{bb_}")
                nc.tensor.matmul(
                    pc[:],
                    mulT_r[0:S, bb_, :],
                    smap_sb[0:S, bb_ * CV : (bb_ + 1) * CV],
                    start=True,
                    stop=True,
                )
                cpo = stgp.tile([64, CV], f32, tag="cpo", name=f"cpo{bb_}")
                nc.vector.tensor_copy(cpo[:], pc[:])
                nc.gpsimd.dma_start(out_r[bb_, :, VREAL : VREAL + CV], cpo[:])

    _split_multi_waits(nc)
    return nc


def _get_nc():
    if "nc" not in _CACHE:
        _CACHE["nc"] = _build_nc()
    return _CACHE["nc"]


def kernel(**inputs):
    from concourse.bass_utils import run_bass_kernel_spmd

    hidden = np.asarray(inputs["hidden"], np.float32)
    attn = np.asarray(inputs["attn"], np.float32)
    src_map = np.asarray(inputs["src_map"], np.float32)
    W = np.asarray(inputs["W"], np.float32)
    b = np.asarray(inputs["b"], np.float32)
    w_copy = np.asarray(inputs["w_copy"], np.float32)
    b_copy = np.asarray(inputs["b_copy"], np.float32)

    nc = _get_nc()

    hT = np.ascontiguousarray(hidden.T).astype(np.float16)     # [D, N]
    attnT16 = np.ascontiguousarray(attn.T).astype(np.float16)  # [S, N]
    smap16 = np.ascontiguousarray(src_map.reshape(S, B * CV)).astype(np.float16)
    wcp_h = np.ascontiguousarray(w_copy.reshape(KT, 128).T).astype(np.float16)
    bcp_h = np.ascontiguousarray(b_copy.reshape(1, 1))

    in_maps = []
    for c in range(NCORES):
        Wc = np.zeros((D, VSH), np.float16)
        Wc[:, :VREAL] = W[:, c * VREAL : (c + 1) * VREAL].astype(np.float16)
        cm = np.ones((128, 512), np.float16)
        zc = np.zeros((128, 1), np.float32)
        if c == 0:
            Wc[:, PAD_IDX] = 0.0
            cm[:, PAD_IDX] = 0.0
            zc[:] = 1.0
        in_maps.append(
            {
                "hT": hT,
                "Wsh": Wc,
                "attnT": attnT16,
                "smap": smap16,
                "wcp": wcp_h,
                "bcp": bcp_h,
                "cmask": cm,
                "zcorr": zc,
            }
        )

    res = run_bass_kernel_spmd(nc, in_maps, list(range(NCORES)), trace=TRACE)
    _CACHE["last_result"] = res

    outs = [r["out"] for r in res.results]
    full = np.empty((N, V + CV), np.float32)
    for c in range(NCORES):
        full[:, c * VREAL : (c + 1) * VREAL] = outs[c][:, :VREAL]
    full[:, V:] = outs[0][:, VREAL:]
    return full



# revision 3
# speedup vs baseline: 2.1256x; 2.1256x over previous
"""CopyGenerator kernel for 8 trn2 NeuronCores.

Strategy (vocab tensor-parallel, fp8 DoubleRow):
  - W's vocab dim (50000) is sharded 6250 cols/core, padded to 6272.
  - Per core: logits = hidden @ (32*W_shard) via PE in fp8e4 with
    perf_mode=DoubleRow (two 128-deep k-slices per matmul, 2x fp8 rate),
    fp32 PSUM accumulate over 4 k-pairs. exp via ACT with scale=1/32 and
    fused row-sum accumulation (exp kept fp16 in SBUF). Softmax denominator
    completed with one tiny AllReduce per row-chunk, then one in-place DVE
    scale pass applies (1-copy)/Z and the tile is stored as one big fp16 DMA.
  - Rows processed in 3 chunks (5,6,5 n-tiles of 128 rows) so chunk i+1's
    matmuls hide chunk i's AllReduce + scale + store.
  - The copy gate sigmoid(hidden@w_copy) is folded in on the host: the host
    passes (1-c) transposed per-partition and attn*c pre-multiplied, so the
    device only runs the einsum vs src_map (32 small fp16 matmuls).
  - DMA queues: W on sync, hT/small on scalar, copy-path stores on vector,
    main stores + collective on gpsimd.
PAD col handling: host zeroes W[:,1] on core 0, kernel masks the exp column
and subtracts the constant exp(0)=1 from that core's row sums.
Output is fp16 on device; host upcasts to fp32.
"""

import numpy as np

N, D, V = 2048, 1024, 50000
S, B, CV = 100, 32, 120
NCORES = 8
VREAL = 6250          # real vocab cols per core
VPAD = 6272           # padded (12*512 + 128)
KT = 8                # k-tiles of 128 over D
JP = KT // 2          # DoubleRow k-pairs
NT = N // 128         # 16 row tiles
CHUNKS = [(0, 5), (5, 6), (11, 5)]   # (first n-tile, n-tile count)
# v-tiles: (col offset, matmul width, exp width)
GTILES = [(i * 512, 512, 512) for i in range(12)] + [(6144, 128, 106)]
PAD_IDX = 1
WSCALE = 32.0

_CACHE = {}
TRACE = False


def _install_walrus_compat():
    """This container's walrus build rejects >1 sync-wait per instruction.
    Patch the Tile drain to chain single-wait drains, and provide a module
    post-pass hoisting extra waits onto same-engine NoOps."""
    import concourse.tile as tile_mod
    import concourse.mybir as mybir
    from concourse.vector_clock import ScopedClock

    if getattr(tile_mod.TileContext._drain_and_barrier, "_waitsplit", False):
        return

    def _patched_drain_and_barrier(self, tick_clock, wait_clock):
        nc = self.nc
        drain_inst = nc.sync.drain()
        wait_clock.add_sem_waits(
            drain_inst.ins, ScopedClock({None: tick_clock.global_clock})
        )
        si = drain_inst.ins.sync_info
        waits = list(si.on_wait) if si and si.on_wait else []
        if len(waits) > 1:
            si.on_wait = waits[:1]
            rest = waits[1:]
            while rest:
                chunk, rest = rest[:1], rest[1:]
                d2 = nc.sync.drain()
                if d2.ins.sync_info is None:
                    d2.ins.sync_info = mybir.SyncInfo(on_wait=chunk, on_update=[])
                else:
                    d2.ins.sync_info.on_wait = chunk
        nc.all_engine_barrier()
        assert self.sems is not None
        popped = nc._tile_sem_poison_stack.pop()
        assert popped is self._sem_poison
        nc.clear_and_free_semaphores(list(self.sems.allocated().values()))
        nc.all_engine_barrier()

    _patched_drain_and_barrier._waitsplit = True
    tile_mod.TileContext._drain_and_barrier = _patched_drain_and_barrier


def _split_multi_waits(nc):
    import concourse.mybir as mybir

    uid = 0
    n_split = 0
    for fn in nc.m.functions:
        for bb in fn.blocks:
            old = list(bb.instructions)
            new = []
            changed = False
            for ins in old:
                si = ins.sync_info
                waits = list(si.on_wait) if si and si.on_wait else []
                if len(waits) > 1:
                    changed = True
                    n_split += 1
                    for w in waits[:-1]:
                        uid += 1
                        new.append(
                            mybir.InstNoOp(
                                name=f"I-waitsplit-{uid}-{ins.name}",
                                sync_info=mybir.SyncInfo(on_wait=[w], on_update=[]),
                                bass_nofuse=True,
                                engine=ins.engine,
                            )
                        )
                    si.on_wait = [waits[-1]]
                new.append(ins)
            if changed:
                bb.instructions[:] = new
    return n_split


def _build_nc():
    import concourse.bass as bass
    import concourse.mybir as mybir
    import concourse.tile as tile

    _install_walrus_compat()

    f32 = mybir.dt.float32
    f16 = mybir.dt.float16
    f8 = mybir.dt.float8e4
    AF = mybir.ActivationFunctionType
    OP = mybir.AluOpType
    AX = mybir.AxisListType
    DR = mybir.MatmulPerfMode.DoubleRow

    nc = bass.Bass()
    hT8 = nc.dram_tensor("hT8", [D, N], f8, kind="ExternalInput")
    W8 = nc.dram_tensor("W8", [D, VPAD], f8, kind="ExternalInput")
    mulT16 = nc.dram_tensor("mulT16", [S, N], f16, kind="ExternalInput")
    smap = nc.dram_tensor("smap", [S, B * CV], f16, kind="ExternalInput")
    omcT = nc.dram_tensor("omcT", [128, NT], f32, kind="ExternalInput")
    cmask = nc.dram_tensor("cmask", [128, 512], f16, kind="ExternalInput")
    zcorr = nc.dram_tensor("zcorr", [128, 1], f32, kind="ExternalInput")
    out = nc.dram_tensor("out", [N, VREAL + CV], f16, kind="ExternalOutput")

    hT8_r = hT8.rearrange("(k p) n -> p k n", p=128)
    W8_r = W8.rearrange("(k p) v -> p k v", p=128)
    MAXNT = max(cnt for _, cnt in CHUNKS)

    with tile.TileContext(nc) as tc:
        with (
            tc.tile_pool(name="htp", bufs=2) as htp,
            tc.tile_pool(name="wp", bufs=3) as wp,
            tc.tile_pool(name="expp", bufs=11) as expp,
            tc.tile_pool(name="zpp", bufs=2 * MAXNT) as zpp,
            tc.tile_pool(name="cpop", bufs=4) as cpop,
            tc.tile_pool(name="smallp", bufs=1) as smallp,
            tc.tile_pool(name="psmain", bufs=6, space="PSUM") as psmain,
            tc.tile_pool(name="psaux", bufs=2, space="PSUM") as psaux,
            tc.tile_pool(name="dramp", bufs=1, space="DRAM") as dramp,
        ):
            # ---- persistent small tiles (scalar-queue loads) ----
            cmask_sb = smallp.tile([128, 512], f16)
            nc.scalar.dma_start(cmask_sb[:], cmask[:])
            zcorr_sb = smallp.tile([128, 1], f32)
            nc.scalar.dma_start(zcorr_sb[:], zcorr[:])
            omcT_sb = smallp.tile([128, NT], f32)
            nc.scalar.dma_start(omcT_sb[:], omcT[:])
            mulT_sb = smallp.tile([128, N], f16)
            nc.scalar.dma_start(mulT_sb[0:S, :], mulT16[:, :])
            smap_sb = smallp.tile([128, B * CV], f16)
            nc.scalar.dma_start(smap_sb[0:S, :], smap[:, :])

            zin = [
                dramp.tile([128, cnt], f32, name=f"zin{ci}")
                for ci, (_, cnt) in enumerate(CHUNKS)
            ]
            zout = [
                dramp.tile([128, cnt], f32, addr_space="Shared", name=f"zout{ci}")
                for ci, (_, cnt) in enumerate(CHUNKS)
            ]

            # ---- copy path first: copy_prob = einsum(attn*copy, src_map) ----
            mulT_r = mulT_sb.rearrange("p (t b) -> p b t", b=B)
            out_r = out[:, :].rearrange("(t b) v -> b t v", b=B)
            for bb_ in range(B):
                pc = psaux.tile([64, CV], f32, tag="psaux", name=f"pc{bb_}")
                nc.tensor.matmul(
                    pc[:],
                    mulT_r[0:S, bb_, :],
                    smap_sb[0:S, bb_ * CV : (bb_ + 1) * CV],
                    start=True,
                    stop=True,
                )
                cpo = cpop.tile([64, CV], f16, tag="cpo", name=f"cpo{bb_}")
                nc.vector.tensor_copy(cpo[:], pc[:])
                nc.sync.dma_start(out_r[bb_, :, VREAL : VREAL + CV], cpo[:])

            # ---- main chunks ----
            for ci, (t0, NTC) in enumerate(CHUNKS):
                ncols = NTC * 128          # rows of this chunk
                c0 = t0 * 128              # first row
                # hidden^T chunk: [p, k, n] layout for DoubleRow pairs
                htc = htp.tile([128, KT, MAXNT * 128], f8, tag="ht", name=f"ht{ci}")
                nc.scalar.dma_start(
                    htc[:, :, 0:ncols], hT8_r[0:128, 0:KT, c0 : c0 + ncols]
                )

                exps = [
                    expp.tile([128, VREAL], f16, tag="exp", name=f"exp{ci}_{t}")
                    for t in range(NTC)
                ]
                zparts = [
                    zpp.tile([128, len(GTILES)], f32, tag="zpart", name=f"zp{ci}_{t}")
                    for t in range(NTC)
                ]
                for gi, (goff, gw, ew) in enumerate(GTILES):
                    wt = wp.tile([128, KT, 512], f8, tag="wt", name=f"wt{ci}_{gi}")
                    nc.sync.dma_start(
                        wt[:, :, 0:gw], W8_r[0:128, 0:KT, goff : goff + gw]
                    )
                    for t in range(NTC):
                        pm = psmain.tile(
                            [128, 512], f32, tag="psmain", name=f"pm{ci}_{gi}_{t}"
                        )
                        for j in range(JP):
                            nc.tensor.matmul(
                                pm[:, 0:gw],
                                htc[:, 2 * j : 2 * j + 2, t * 128 : (t + 1) * 128],
                                wt[:, 2 * j : 2 * j + 2, 0:gw],
                                start=(j == 0),
                                stop=(j == JP - 1),
                                perf_mode=DR,
                            )
                        nc.scalar.activation(
                            exps[t][:, goff : goff + ew], pm[:, 0:ew], AF.Exp,
                            scale=1.0 / WSCALE,
                            accum_out=zparts[t][:, gi : gi + 1],
                        )
                        if gi == 0:
                            # zero masked cols (PAD on core 0; all-ones elsewhere)
                            nc.vector.tensor_tensor(
                                exps[t][:, 0:512], exps[t][:, 0:512], cmask_sb[:],
                                OP.mult,
                            )

                # ---- denominator: reduce partials, AllReduce across cores ----
                zsum = smallp.tile([128, NTC], f32, name=f"zsum{ci}")
                for t in range(NTC):
                    nc.vector.tensor_reduce(
                        zsum[:, t : t + 1], zparts[t][:, :], axis=AX.X, op=OP.add
                    )
                nc.vector.tensor_scalar(
                    zsum[:], zsum[:], zcorr_sb[:], None, OP.subtract
                )
                nc.gpsimd.dma_start(zin[ci][:], zsum[:])
                nc.gpsimd.collective_compute(
                    "AllReduce",
                    OP.add,
                    ins=[zin[ci].opt()],
                    outs=[zout[ci].opt()],
                    replica_groups=[list(range(NCORES))],
                )
                zr = smallp.tile([128, NTC], f32, name=f"zr{ci}")
                nc.scalar.dma_start(zr[:], zout[ci][:])
                rz = smallp.tile([128, NTC], f32, name=f"rz{ci}")
                nc.vector.reciprocal(rz[:], zr[:])
                sc = smallp.tile([128, NTC], f32, name=f"sc{ci}")
                nc.vector.tensor_tensor(
                    sc[:], omcT_sb[:, t0 : t0 + NTC], rz[:], OP.mult
                )

                # ---- pass 2: in-place scale on DVE, one big store per tile ----
                for t in range(NTC):
                    r0 = (t0 + t) * 128
                    nc.vector.tensor_scalar(
                        exps[t][:, 0:VREAL],
                        exps[t][:, 0:VREAL],
                        sc[:, t : t + 1],
                        None,
                        OP.mult,
                    )
                    nc.gpsimd.dma_start(
                        out[r0 : r0 + 128, 0:VREAL], exps[t][:, 0:VREAL]
                    )

    _split_multi_waits(nc)
    return nc


def _get_nc():
    if "nc" not in _CACHE:
        _CACHE["nc"] = _build_nc()
    return _CACHE["nc"]


def kernel(**inputs):
    import ml_dtypes
    from concourse.bass_utils import run_bass_kernel_spmd

    f8np = ml_dtypes.float8_e4m3

    hidden = np.asarray(inputs["hidden"], np.float32)
    attn = np.asarray(inputs["attn"], np.float32)
    src_map = np.asarray(inputs["src_map"], np.float32)
    W = np.asarray(inputs["W"], np.float32)
    w_copy = np.asarray(inputs["w_copy"], np.float32)
    b_copy = np.asarray(inputs["b_copy"], np.float32)

    nc = _get_nc()

    # host-side copy gate (tiny: N x D @ D x 1)
    c = 1.0 / (1.0 + np.exp(-(hidden @ w_copy + b_copy)))      # [N, 1] f32
    omc = (1.0 - c[:, 0]).astype(np.float32)                   # [N]
    omcT_h = np.ascontiguousarray(omc.reshape(NT, 128).T)      # [128, NT]
    mulT_h = np.ascontiguousarray((attn * c).T).astype(np.float16)  # [S, N]

    hT8_h = np.ascontiguousarray(hidden.T).astype(f8np)        # [D, N]
    smap16 = np.ascontiguousarray(src_map.reshape(S, B * CV)).astype(np.float16)

    in_maps = []
    for cc in range(NCORES):
        Wc = np.zeros((D, VPAD), f8np)
        Wcf = W[:, cc * VREAL : (cc + 1) * VREAL] * WSCALE
        if cc == 0:
            Wcf = Wcf.copy()
            Wcf[:, PAD_IDX] = 0.0
        Wc[:, :VREAL] = Wcf.astype(f8np)
        cm = np.ones((128, 512), np.float16)
        zc = np.zeros((128, 1), np.float32)
        if cc == 0:
            cm[:, PAD_IDX] = 0.0
            zc[:] = 1.0
        in_maps.append(
            {
                "hT8": hT8_h,
                "W8": Wc,
                "mulT16": mulT_h,
                "smap": smap16,
                "omcT": omcT_h,
                "cmask": cm,
                "zcorr": zc,
            }
        )

    res = run_bass_kernel_spmd(nc, in_maps, list(range(NCORES)), trace=TRACE)
    _CACHE["last_result"] = res

    outs = [r["out"] for r in res.results]
    full = np.empty((N, V + CV), np.float32)
    for cc in range(NCORES):
        full[:, cc * VREAL : (cc + 1) * VREAL] = outs[cc][:, :VREAL]
    full[:, V:] = outs[0][:, VREAL:]
    return full
